# revision 3
# baseline (speedup 1.0000x reference)
"""HMQ-quantized MLP (fc1 -> exact GELU -> fc2) on 8 TRN2 NeuronCores.

Strategy: data-parallel over the 16384 token rows (2048 rows/core).
The int8 fake-quant values are integers in [-127, 127], exactly representable
in bf16, and all dot-product partial sums stay far below 2^24 -- so the
dequantized GEMMs are computed EXACTLY as bf16 integer matmuls on the PE
array (full 78.6 TF/s rate) with fp32 PSUM accumulation, then scaled by
s_a*s_w.  Rounding uses the +/-1.5*2^23 magic-constant trick (matches
jnp.round's round-half-to-even bit-exactly; validated on HW).

Per-tensor global maxes need cross-core reduction: each core reduces its
x-shard / w1-shard / w2-shard, then one tiny AllReduce(max) supplies the
fc1-side scales; a second AllReduce(max) after GELU supplies the hidden
scale.  gelu(h) is staged to DRAM transposed (fc1 computes h^T directly) so
fc2 needs no on-chip transposes anywhere (x^T, w1^T, w2^T are laid out on
the host as part of sharding).
"""

import numpy as np

import concourse.bass as bass
import concourse.mybir as mybir
import concourse.tile as tile
from concourse import bacc, bass_isa
from concourse.bass_utils import run_bass_kernel_spmd

F32 = mybir.dt.float32
BF16 = mybir.dt.bfloat16
ts = bass.ts

C_MAGIC = 1.5 * 2**23  # round-to-nearest-even for |v| < 2^22
QMAX = 127.0

NCORES = 8
B, T, D, H = 4, 4096, 1024, 4096
M = B * T            # 16384 total rows
S = M // NCORES      # 2048 rows per core

N_IC = D // 128      # 8  contraction chunks for fc1
N_OC = H // 128      # 32 output chunks for fc1 (hidden)
N_ST = S // 512      # 4  row tiles of 512
N_SC = S // 128      # 16 row chunks of 128
N_NC = H // 128      # 32 contraction chunks for fc2
N_JT = D // 512      # 2  output col tiles for fc2

Copy = mybir.ActivationFunctionType.Copy
Gelu = mybir.ActivationFunctionType.Gelu
X_AX = mybir.AxisListType.X
MAX = mybir.AluOpType.max
MULT = mybir.AluOpType.mult
SUB = mybir.AluOpType.subtract
ADD = mybir.AluOpType.add


def _quantize(nc, pool_out_slice, stage_ap, inv_scale_ap):
    """stage (f32, in-place) -> round(stage*inv_scale) -> bf16 out slice."""
    nc.scalar.activation(stage_ap, stage_ap, Copy, bias=C_MAGIC, scale=inv_scale_ap)
    nc.vector.tensor_scalar(out=pool_out_slice, in0=stage_ap, scalar1=C_MAGIC,
                            scalar2=None, op0=SUB)


def build():
    nc = bacc.Bacc("TRN2", target_bir_lowering=False, debug=False,
                   num_devices=NCORES)

    xts = nc.dram_tensor("xts", [D, S], F32, kind="ExternalInput")
    w1t = nc.dram_tensor("w1t", [D, H], F32, kind="ExternalInput")
    w1s = nc.dram_tensor("w1s", [H // NCORES, D], F32, kind="ExternalInput")
    w2t = nc.dram_tensor("w2t", [H, D], F32, kind="ExternalInput")
    w2s = nc.dram_tensor("w2s", [D // NCORES, H], F32, kind="ExternalInput")
    b1m = nc.dram_tensor("b1m", [H // 128, 128], F32, kind="ExternalInput")
    b2m = nc.dram_tensor("b2m", [1, D], F32, kind="ExternalInput")
    out = nc.dram_tensor("out", [S, D], F32, kind="ExternalOutput")

    with tile.TileContext(nc) as tc:
        with (
            tc.tile_pool(name="misc", bufs=1) as misc,
            tc.tile_pool(name="stage", bufs=2) as stagep,
            tc.tile_pool(name="xq", bufs=1) as xqp,
            tc.tile_pool(name="w1stage", bufs=3) as w1sp,
            tc.tile_pool(name="w1q", bufs=3) as w1qp,
            tc.tile_pool(name="w2stage", bufs=3) as w2sp,
            tc.tile_pool(name="w2q", bufs=1) as w2qp,
            tc.tile_pool(name="gout", bufs=4) as goutp,
            tc.tile_pool(name="gq", bufs=2) as gqp,
            tc.tile_pool(name="outp", bufs=3) as outp,
            tc.tile_pool(name="psum", bufs=8, space="PSUM") as psump,
            tc.tile_pool(name="dram", bufs=1, space="DRAM") as dramp,
        ):
            # ---------------- persistent DRAM intermediates ----------------
            gT = dramp.tile([H, S], F32, tag="gT")
            cc1_in = dramp.tile([1, 4], F32, tag="cc1i")
            cc1_out = dramp.tile([1, 4], F32, tag="cc1o")
            cc2_in = dramp.tile([1, 4], F32, tag="cc2i")
            cc2_out = dramp.tile([1, 4], F32, tag="cc2o")

            # ---------------- bias prep ----------------
            id32 = misc.tile([32, 32], F32, tag="id32")
            from concourse.masks import make_identity
            make_identity(nc, id32)
            b1sb = misc.tile([H // 128, 128], F32, tag="b1sb")
            nc.sync.dma_start(out=b1sb, in_=b1m[:, :])
            b1ps = psump.tile([128, H // 128], F32, tag="mm")
            nc.tensor.transpose(b1ps, b1sb, id32)
            b1all = misc.tile([128, H // 128], F32, tag="b1all")
            nc.vector.tensor_copy(b1all, b1ps)

            b2row = misc.tile([1, D], F32, tag="b2row")
            nc.sync.dma_start(out=b2row, in_=b2m[:, :])
            b2r = misc.tile([128, D], F32, tag="b2r")
            nc.gpsimd.partition_broadcast(b2r, b2row)

            # ---------------- local abs-max reductions ----------------
            # x shard: 4 big chunks of [128, 2*S]
            part1 = misc.tile([128, 8], F32, tag="part1")
            for c in range(4):
                xc = stagep.tile([128, 2, S], F32, tag="stage", name=f"xmax{c}")
                nc.sync.dma_start(
                    out=xc,
                    in_=xts[c * 256:(c + 1) * 256, :].rearrange(
                        "(a p) s -> p a s", p=128),
                )
                nc.vector.tensor_reduce(out=part1[:, c:c + 1], in_=xc,
                                        axis=mybir.AxisListType.XY,
                                        op=MAX, apply_absolute_value=True)
            # w1 shard [512, 1024] -> [128, 4, 1024]
            w1sc = stagep.tile([128, 4, D], F32, tag="stage", name="w1maxc")
            nc.sync.dma_start(
                out=w1sc,
                in_=w1s[:, :].rearrange("(a p) d -> p a d", p=128))
            nc.vector.tensor_reduce(out=part1[:, 4:5], in_=w1sc,
                                    axis=mybir.AxisListType.XY,
                                    op=MAX, apply_absolute_value=True)
            # w2 shard [128, 4096]
            w2sc = stagep.tile([128, 4096], F32, tag="stage", name="w2maxc")
            nc.sync.dma_start(out=w2sc, in_=w2s[:, :])
            nc.vector.tensor_reduce(out=part1[:, 5:6], in_=w2sc, axis=X_AX,
                                    op=MAX, apply_absolute_value=True)

            # combine x partials -> col0, fold into [128,3] row block
            arow = misc.tile([128, 4], F32, tag="arow")
            nc.vector.tensor_reduce(out=arow[:, 0:1], in_=part1[:, 0:4], axis=X_AX,
                                    op=MAX)
            nc.vector.tensor_copy(arow[:, 1:3], part1[:, 4:6])
            nc.vector.tensor_copy(arow[:, 3:4], part1[:, 5:6])
            armax = misc.tile([128, 4], F32, tag="armax")
            nc.gpsimd.partition_all_reduce(armax, arow, channels=128,
                                           reduce_op=bass_isa.ReduceOp.max)

            # ---------------- AllReduce #1: global Mx, Mw1, Mw2 ----------------
            nc.sync.dma_start(out=cc1_in, in_=armax[0:1, :])
            nc.gpsimd.collective_compute(
                "AllReduce", MAX, replica_groups=[list(range(NCORES))],
                ins=[cc1_in.opt()], outs=[cc1_out.opt()])
            g1row = misc.tile([1, 4], F32, tag="g1row")
            nc.sync.dma_start(out=g1row, in_=cc1_out[:, :])
            g1 = misc.tile([128, 4], F32, tag="g1")
            nc.gpsimd.partition_broadcast(g1, g1row)

            # scl cols: 0 sx | 1 inv_sx | 2 sw1 | 3 inv_sw1 | 4 sw2 | 5 inv_sw2 | 6 d1
            scl = misc.tile([128, 8], F32, tag="scl")
            for i in range(3):
                nc.vector.tensor_scalar(out=scl[:, 2 * i:2 * i + 1],
                                        in0=g1[:, i:i + 1],
                                        scalar1=1e-8, scalar2=1.0 / QMAX,
                                        op0=MAX, op1=MULT)
                nc.vector.reciprocal(scl[:, 2 * i + 1:2 * i + 2],
                                     scl[:, 2 * i:2 * i + 1])
            nc.vector.tensor_mul(scl[:, 6:7], scl[:, 0:1], scl[:, 2:3])

            # ---------------- quantize x -> xqT (bf16, resident) ----------------
            xqT = xqp.tile([128, N_IC, S], BF16, tag="xq")
            for ic in range(N_IC):
                xc = stagep.tile([128, S], F32, tag="stage", name=f"xq{ic}")
                nc.sync.dma_start(out=xc, in_=xts[ic * 128:(ic + 1) * 128, :])
                _quantize(nc, xqT[:, ic, :], xc, scl[:, 1:2])

            # ---------------- fc1: h^T = w1q @ xq^T, gelu, stage g^T ----------
            gpart = misc.tile([128, N_OC * N_ST], F32, tag="gpart")
            for oc in range(N_OC):
                w1c = w1sp.tile([128, N_IC, 128], F32, tag="w1c")
                nc.sync.dma_start(
                    out=w1c,
                    in_=w1t[:, ts(oc, 128)].rearrange("(ic p) o -> p ic o", p=128))
                w1q = w1qp.tile([128, N_IC, 128], BF16, tag="w1q")
                _quantize(nc, w1q.rearrange("p a b -> p (a b)"),
                          w1c.rearrange("p a b -> p (a b)"), scl[:, 3:4])
                pts = [psump.tile([128, 512], F32, tag="mm", name=f"pt{oc}_{st}")
                       for st in range(N_ST)]
                for ic in range(N_IC):
                    for st in range(N_ST):
                        nc.tensor.matmul(pts[st], lhsT=w1q[:, ic, :],
                                         rhs=xqT[:, ic, ts(st, 512)],
                                         start=(ic == 0), stop=(ic == N_IC - 1))
                for st in range(N_ST):
                    go = goutp.tile([128, 512], F32, tag="gout",
                                    name=f"go{oc}_{st}")
                    nc.scalar.activation(go, pts[st], Gelu,
                                         bias=b1all[:, oc:oc + 1],
                                         scale=scl[:, 6:7])
                    nc.vector.tensor_reduce(
                        out=gpart[:, oc * N_ST + st:oc * N_ST + st + 1],
                        in_=go, axis=X_AX, op=MAX, apply_absolute_value=True)
                    nc.sync.dma_start(out=gT[ts(oc, 128), ts(st, 512)], in_=go)

            # ---------------- quantize w2 -> w2qT (bf16, resident) -------------
            w2qT = w2qp.tile([128, N_NC, D], BF16, tag="w2q")
            for nc_ in range(N_NC):
                w2c = w2sp.tile([128, D], F32, tag="w2c")
                nc.sync.dma_start(out=w2c, in_=w2t[ts(nc_, 128), :])
                _quantize(nc, w2qT[:, nc_, :], w2c, scl[:, 5:6])

            # ---------------- AllReduce #2: global Mg ----------------
            garow = misc.tile([128, 4], F32, tag="garow")
            nc.vector.tensor_reduce(out=garow[:, 0:1], in_=gpart, axis=X_AX, op=MAX)
            for j in range(1, 4):
                nc.vector.tensor_copy(garow[:, j:j + 1], garow[:, 0:1])
            gamax = misc.tile([128, 4], F32, tag="gamax")
            nc.gpsimd.partition_all_reduce(gamax, garow, channels=128,
                                           reduce_op=bass_isa.ReduceOp.max)
            nc.sync.dma_start(out=cc2_in, in_=gamax[0:1, :])
            nc.gpsimd.collective_compute(
                "AllReduce", MAX, replica_groups=[list(range(NCORES))],
                ins=[cc2_in.opt()], outs=[cc2_out.opt()])
            g2row = misc.tile([1, 4], F32, tag="g2row")
            nc.sync.dma_start(out=g2row, in_=cc2_out[:, :])
            g2 = misc.tile([128, 4], F32, tag="g2")
            nc.gpsimd.partition_broadcast(g2, g2row)

            # scl2 cols: 0 sg | 1 inv_sg | 2 d2
            scl2 = misc.tile([128, 4], F32, tag="scl2")
            nc.vector.tensor_scalar(out=scl2[:, 0:1], in0=g2[:, 0:1],
                                    scalar1=1e-8, scalar2=1.0 / QMAX,
                                    op0=MAX, op1=MULT)
            nc.vector.reciprocal(scl2[:, 1:2], scl2[:, 0:1])
            nc.vector.tensor_mul(scl2[:, 2:3], scl2[:, 0:1], scl[:, 4:5])

            # ---------------- fc2: out = gq^T.T @ w2q^T ----------------
            for sc in range(N_SC):
                gts = stagep.tile([128, N_NC, 128], F32, tag="stage",
                                  name=f"gts{sc}")
                nc.sync.dma_start(
                    out=gts,
                    in_=gT[:, ts(sc, 128)].rearrange("(a p) s -> p a s", p=128))
                gq = gqp.tile([128, N_NC, 128], BF16, tag="gq")
                _quantize(nc, gq.rearrange("p a b -> p (a b)"),
                          gts.rearrange("p a b -> p (a b)"), scl2[:, 1:2])
                pos = [psump.tile([128, 512], F32, tag="mm", name=f"po{sc}_{jt}")
                       for jt in range(N_JT)]
                for nc_ in range(N_NC):
                    for jt in range(N_JT):
                        nc.tensor.matmul(pos[jt], lhsT=gq[:, nc_, :],
                                         rhs=w2qT[:, nc_, ts(jt, 512)],
                                         start=(nc_ == 0), stop=(nc_ == N_NC - 1))
                for jt in range(N_JT):
                    ot = outp.tile([128, 512], F32, tag="ot", name=f"ot{sc}_{jt}")
                    nc.scalar.activation(ot, pos[jt], Copy, bias=0.0,
                                         scale=scl2[:, 2:3])
                    nc.vector.tensor_add(ot, ot, b2r[:, ts(jt, 512)])
                    nc.sync.dma_start(out=out[ts(sc, 128), ts(jt, 512)], in_=ot)

    nc.compile()
    return nc


_NC_CACHE = None


def _get_nc():
    global _NC_CACHE
    if _NC_CACHE is None:
        _NC_CACHE = build()
    return _NC_CACHE


def make_in_maps(x, w1, b1, w2, b2):
    xf = np.ascontiguousarray(x.reshape(M, D).T)          # [D, M]
    w1t_h = np.ascontiguousarray(w1.T)                    # [D, H]
    w2t_h = np.ascontiguousarray(w2.T)                    # [H, D]
    b1m_h = np.ascontiguousarray(b1.reshape(H // 128, 128))
    b2m_h = np.ascontiguousarray(b2.reshape(1, D))
    in_maps = []
    for c in range(NCORES):
        in_maps.append({
            "xts": np.ascontiguousarray(xf[:, c * S:(c + 1) * S]),
            "w1t": w1t_h,
            "w1s": np.ascontiguousarray(w1[c * (H // NCORES):(c + 1) * (H // NCORES), :]),
            "w2t": w2t_h,
            "w2s": np.ascontiguousarray(w2[c * (D // NCORES):(c + 1) * (D // NCORES), :]),
            "b1m": b1m_h,
            "b2m": b2m_h,
        })
    return in_maps


def kernel(x, w1, b1, w2, b2, _trace=False):
    nc = _get_nc()
    in_maps = make_in_maps(np.asarray(x, dtype=np.float32),
                           np.asarray(w1, dtype=np.float32),
                           np.asarray(b1, dtype=np.float32),
                           np.asarray(w2, dtype=np.float32),
                           np.asarray(b2, dtype=np.float32))
    res = run_bass_kernel_spmd(nc, in_maps, core_ids=list(range(NCORES)),
                               trace=_trace)
    full = np.concatenate([res.results[c]["out"] for c in range(NCORES)], axis=0)
    out = full.reshape(B, T, D)
    if _trace:
        kernel.last_results = res
    return out
